# revision 3
# baseline (speedup 1.0000x reference)
"""GNN message passing (out = A @ x, A[src,dst] = edge_attr) on 8 TRN2 NeuronCores.

Strategy: shard by destination segment (src rows of the output) across 8 cores.
Each core owns a contiguous 12,500-node src range and the edges whose src falls
in it. Per core:
  - edges are binned into (src-block [128 nodes], dst-bucket [25,000 nodes]) cells
  - a uniform chunk count per cell (max over cores) makes one static program
    serve all 8 cores (SPMD)
  - x rows are fetched with the SWDGE dma_gather custom op (bf16, 64B payload,
    256B stride, int16 indices local to the dst bucket)
  - DVE builds a one-hot scatter matrix per 128-edge chunk (iota == src_local)
    and folds edge weights into the gathered rows
  - PE matmul (one-hot^T @ messages) accumulates each src-block's [128, 32]
    output tile directly in PSUM across all of the block's chunks
  - one DVE copy PSUM->SBUF and one DMA writes the core's whole output
"""

import sys

sys.path.insert(0, "/opt/trn_rl_repo")

import numpy as np
import ml_dtypes

import concourse.bacc as bacc
import concourse.bass as bass
import concourse.mybir as mybir
from concourse.library_config import mlp
from concourse import bass_utils

N_NODES = 100000
D_FEAT = 32
N_CORES = 8
SRC_PER_CORE = N_NODES // N_CORES          # 12500
BLOCK = 128                                 # src nodes per block
N_BLOCKS = (SRC_PER_CORE + BLOCK - 1) // BLOCK   # 98
N_BUCKETS = 4
BUCKET = N_NODES // N_BUCKETS               # 25000 (fits int16 token index)
XPAD = 128                                  # bf16 row padded to 256B stride
NB = 64                                     # chunks per gather call / batch
NBUF = 3                                    # G/W buffer rotation depth

LAST_RESULTS = None                         # set by kernel() for test.py


def _build_host_data(edge_index, edge_attr):
    src = np.asarray(edge_index[0], dtype=np.int64)
    dst = np.asarray(edge_index[1], dtype=np.int64)
    w = np.asarray(edge_attr, dtype=np.float32)
    E = src.shape[0]

    core = src // SRC_PER_CORE
    sloc = src % SRC_PER_CORE
    b = sloc // BLOCK
    srcl = sloc - b * BLOCK                  # 0..127 within block
    q = dst // BUCKET
    dstl = dst - q * BUCKET                  # 0..24999 within bucket

    # per (core, q, b) cell counts -> shared uniform chunk counts K[q, b]
    cell = (core * N_BUCKETS + q) * N_BLOCKS + b
    counts = np.bincount(cell, minlength=N_CORES * N_BUCKETS * N_BLOCKS)
    counts = counts.reshape(N_CORES, N_BUCKETS, N_BLOCKS)
    K = -(-counts.max(axis=0) // BLOCK)      # [N_BUCKETS, N_BLOCKS]
    K[0] = np.maximum(K[0], 1)               # every block writes its PSUM region

    chunk_start = np.zeros((N_BUCKETS, N_BLOCKS), dtype=np.int64)
    flat = K.reshape(-1)
    chunk_start.reshape(-1)[1:] = np.cumsum(flat)[:-1]
    C = int(flat.sum())

    # schedule metadata per chunk: bucket, block, start, stop
    chunk_q = np.repeat(np.arange(N_BUCKETS)[:, None], N_BLOCKS, 1).reshape(-1)
    chunk_q = np.repeat(chunk_q, flat)
    chunk_b = np.repeat(np.tile(np.arange(N_BLOCKS), N_BUCKETS), flat)
    is_start = np.zeros(C, dtype=bool)
    is_stop = np.zeros(C, dtype=bool)
    for bb in range(N_BLOCKS):
        own = np.where(chunk_b == bb)[0]
        is_start[own[0]] = True
        is_stop[own[-1]] = True

    # per-core slot assignment (slot = chunk*128 + lane)
    order = np.argsort(cell, kind="stable")
    cs = np.bincount(cell, minlength=N_CORES * N_BUCKETS * N_BLOCKS)
    cell_first = np.zeros_like(cs)
    cell_first[1:] = np.cumsum(cs)[:-1]
    rank = np.arange(E) - cell_first[cell[order]]
    slot_base = (chunk_start[q[order], b[order]] * BLOCK)
    slot = slot_base + rank                 # within this edge's core

    per_core = []
    dstl_o = dstl[order]
    srcl_o = srcl[order]
    w_o = w[order]
    core_o = core[order]
    for c in range(N_CORES):
        m = core_o == c
        s = slot[m]
        dl = np.zeros(C * BLOCK, dtype=np.int16)
        sl = np.zeros(C * BLOCK, dtype=np.int16)
        wv = np.zeros(C * BLOCK, dtype=np.float32)
        dl[s] = dstl_o[m].astype(np.int16)
        sl[s] = srcl_o[m].astype(np.int16)
        wv[s] = w_o[m]
        per_core.append((dl, sl, wv))

    # batches: per bucket, runs of <= NB chunks
    batches = []   # (q, cs_chunk, n_chunks)
    pos = 0
    for qq in range(N_BUCKETS):
        nq = int(K[qq].sum())
        done = 0
        while done < nq:
            n = min(NB, nq - done)
            batches.append((qq, pos + done, n))
            done += n
        pos += nq

    # wrapped int16 gather index arrays per core: [128, C*8]
    idx_w_cores = []
    for c in range(N_CORES):
        dl = per_core[c][0]
        cols = []
        for (qq, cs_c, n) in batches:
            flat_idx = dl[cs_c * BLOCK:(cs_c + n) * BLOCK]     # slot order == j order
            wrapped = flat_idx.reshape(-1, 16).T               # [16, ni/16]
            cols.append(np.tile(wrapped, (8, 1)))              # [128, ni/16]
        idx_w_cores.append(np.concatenate(cols, axis=1))

    sched = {
        "C": C,
        "chunk_b": chunk_b,
        "is_start": is_start,
        "is_stop": is_stop,
        "batches": batches,
    }
    return sched, per_core, idx_w_cores


def _dma_gather_raw(gpsimd, nc, out_ap, in_ap, idxs_ap, num_idxs, elem_size,
                    stride_bytes_256):
    """dma_gather with a sub-256B payload (elem_size*dtype < 256B) and an
    explicit 256B-multiple row stride. Same instruction the stock wrapper
    emits; the stock wrapper just over-asserts elem alignment."""
    _in_ap = gpsimd.lower_ap_dma(in_ap, for_custom_bir_dma=True)
    _idxs_ap = gpsimd.lower_ap(idxs_ap)
    _out_ap = gpsimd.lower_ap(out_ap)
    return gpsimd.add_instruction(
        mybir.InstDMAGatherAnt(
            name=nc.get_next_instruction_name(),
            ins=[*_in_ap, _idxs_ap, gpsimd.lower_val_access(gpsimd.to_reg(num_idxs))],
            outs=[_out_ap],
            transpose=False, num_idxs=num_idxs, elem_size=elem_size,
            stride_bytes_256=stride_bytes_256, gen_mode=0, single_packet=False,
            queue_num=0, sbuf_tokens_per_rank=0, sbuf_free_dim_per_rank=0,
            sbuf_free_dim_pad_per_rank=0, sbuf_byte_offset=0,
        )
    )


def _build_program(sched):
    C = sched["C"]
    chunk_b = sched["chunk_b"]
    is_start = sched["is_start"]
    is_stop = sched["is_stop"]
    batches = sched["batches"]
    nbatches = len(batches)
    OUTC = N_BLOCKS * D_FEAT                 # 3136

    bf16 = mybir.dt.bfloat16
    f32 = mybir.dt.float32

    nc = bacc.Bacc("TRN2", target_bir_lowering=False, debug=False,
                   num_devices=N_CORES)
    x_d = nc.dram_tensor("x", [N_NODES, XPAD], bf16, kind="ExternalInput")
    idx_d = nc.dram_tensor("idxw", [128, C * 8], mybir.dt.int16, kind="ExternalInput")
    srcl_d = nc.dram_tensor("srcl", [128, C], bf16, kind="ExternalInput")
    w_d = nc.dram_tensor("w", [128, C], bf16, kind="ExternalInput")
    iota_d = nc.dram_tensor("iota", [128, 128], bf16, kind="ExternalInput")
    out_d = nc.dram_tensor("out", [128, OUTC], f32, kind="ExternalOutput")

    with (
        nc.Block() as block,
        nc.sbuf_tensor("idx_sb", [128, C * 8], mybir.dt.int16) as idx_sb,
        nc.sbuf_tensor("srcl_sb", [128, C], bf16) as srcl_sb,
        nc.sbuf_tensor("w_sb", [128, C], bf16) as w_sb,
        nc.sbuf_tensor("iota_sb", [128, 128], bf16) as iota_sb,
        nc.sbuf_tensor("g_sb", [128, NBUF, NB * D_FEAT], bf16) as g_sb,
        nc.sbuf_tensor("wm_sb", [128, NBUF, NB * 128], bf16) as wm_sb,
        nc.sbuf_tensor("out_sb", [128, OUTC], f32) as out_sb,
        nc.psum_tensor("ps", [128, OUTC], f32) as ps,
        nc.semaphore("io") as io,
        nc.semaphore("gsem0") as gsem0,
        nc.semaphore("gsem1") as gsem1,
        nc.semaphore("gsem2") as gsem2,
        nc.semaphore("wsem") as wsem,
        nc.semaphore("psem") as psem,
        nc.semaphore("fin") as fin,
    ):
        @block.sync
        def _(sync):
            sync.dma_start(idx_sb[:], idx_d[:]).then_inc(io, 16)
            sync.dma_start(srcl_sb[:], srcl_d[:]).then_inc(io, 16)
            sync.dma_start(w_sb[:], w_d[:]).then_inc(io, 16)
            sync.dma_start(iota_sb[:], iota_d[:]).then_inc(io, 16)
            sync.wait_ge(fin, 1)
            sync.dma_start(out_d[:], out_sb[:]).then_inc(io, 16)
            sync.wait_ge(io, 80)

        @block.gpsimd
        def _(gpsimd):
            gpsimd.load_library(mlp)
            gpsimd.wait_ge(io, 64)  # all inputs loaded
            icol = 0
            for i, (qq, cs_c, n) in enumerate(batches):
                if i >= NBUF:
                    gpsimd.wait_ge(psem, i - NBUF + 1)
                ni = n * BLOCK
                buf = i % NBUF
                _dma_gather_raw(
                    gpsimd, nc,
                    out_ap=g_sb[:, buf, :n * D_FEAT].rearrange(
                        "p (n e) -> p n e", e=D_FEAT),
                    in_ap=x_d[qq * BUCKET:(qq + 1) * BUCKET, :D_FEAT],
                    idxs_ap=idx_sb[:, icol:icol + ni // 16],
                    num_idxs=ni, elem_size=D_FEAT,
                    stride_bytes_256=(XPAD * 2) // 256,
                ).then_inc([gsem0, gsem1, gsem2][buf], 16)
                icol += ni // 16

        @block.vector
        def _(vector):
            vector.memset(ps[:], 0.0).then_inc(wsem, 1)
            vector.wait_ge(io, 64)
            for i, (qq, cs_c, n) in enumerate(batches):
                buf = i % NBUF
                if i >= NBUF:
                    vector.wait_ge(psem, i - NBUF + 1)
                w3 = wm_sb[:, buf, :n * 128].rearrange("p (n s) -> p n s", s=128)
                vector.tensor_tensor(
                    out=w3,
                    in0=iota_sb[:, None, :].broadcast_to([128, n, 128]),
                    in1=srcl_sb[:, cs_c:cs_c + n, None].broadcast_to([128, n, 128]),
                    op=mybir.AluOpType.is_equal,
                ).then_inc(wsem, 1)
                vector.wait_ge([gsem0, gsem1, gsem2][buf], 16 * (i // NBUF + 1))
                g3 = g_sb[:, buf, :n * D_FEAT].rearrange("p (n e) -> p n e", e=D_FEAT)
                vector.tensor_tensor(
                    out=g3, in0=g3,
                    in1=w_sb[:, cs_c:cs_c + n, None].broadcast_to([128, n, D_FEAT]),
                    op=mybir.AluOpType.mult,
                ).then_inc(wsem, 1)
            vector.wait_ge(psem, nbatches)
            vector.tensor_copy(out=out_sb[:], in_=ps[:]).then_inc(fin, 1)

        @block.tensor
        def _(tensor):
            for i, (qq, cs_c, n) in enumerate(batches):
                buf = i % NBUF
                tensor.wait_ge(wsem, 2 * (i + 1) + 1)
                for k in range(n):
                    c = cs_c + k
                    off = int(chunk_b[c]) * D_FEAT
                    mm = nc.tensor.matmul(
                        out=ps[:, off:off + D_FEAT],
                        lhsT=wm_sb[:, buf, k * 128:(k + 1) * 128],
                        rhs=g_sb[:, buf, k * D_FEAT:(k + 1) * D_FEAT],
                        start=False, stop=False,
                        skip_group_check=True,
                    )
                mm.then_inc(psem, 1)


    nc.compile()
    return nc


def kernel(edge_index, edge_attr, x):
    sched, per_core, idx_w_cores = _build_host_data(edge_index, edge_attr)
    C = sched["C"]

    x_bf = np.zeros((N_NODES, XPAD), dtype=ml_dtypes.bfloat16)
    x_bf[:, :D_FEAT] = np.asarray(x, dtype=np.float32).astype(ml_dtypes.bfloat16)
    iota = np.tile(np.arange(128, dtype=np.float32).astype(ml_dtypes.bfloat16),
                   (128, 1))

    nc = _build_program(sched)

    in_maps = []
    for c in range(N_CORES):
        dl, sl, wv = per_core[c]
        in_maps.append({
            "x": x_bf,
            "idxw": idx_w_cores[c],
            "srcl": sl.reshape(C, BLOCK).T.astype(ml_dtypes.bfloat16).copy(),
            "w": wv.reshape(C, BLOCK).T.astype(ml_dtypes.bfloat16).copy(),
            "iota": iota,
        })

    res = bass_utils.run_bass_kernel_spmd(nc, in_maps, core_ids=list(range(N_CORES)))
    global LAST_RESULTS
    LAST_RESULTS = res

    out = np.empty((N_NODES, D_FEAT), dtype=np.float32)
    for c in range(N_CORES):
        o = res.results[c]["out"]                      # [128, 98*32]
        o = o.reshape(128, N_BLOCKS, D_FEAT).transpose(1, 0, 2).reshape(-1, D_FEAT)
        out[c * SRC_PER_CORE:(c + 1) * SRC_PER_CORE] = o[:SRC_PER_CORE]
    return out



# revision 9
# speedup vs baseline: 2.1337x; 2.1337x over previous
"""GNN message passing (out = A @ x, A[src,dst] = edge_attr) on 8 TRN2 NeuronCores.

Strategy: shard by destination segment (src rows of the output) across 8 cores.
Each core owns a contiguous 12,500-node src range and the edges whose src falls
in it. Per core:
  - edges are binned into (src-block [128 nodes], dst-bucket [25,000 nodes]) cells
  - a uniform chunk count per cell (max over cores) makes one static program
    serve all 8 cores (SPMD)
  - x rows are fetched with the SWDGE dma_gather custom op (bf16, 64B payload,
    256B stride, int16 indices local to the dst bucket)
  - DVE builds a one-hot scatter matrix per 128-edge chunk (iota == src_local)
    and folds edge weights into the gathered rows
  - PE matmul (one-hot^T @ messages) accumulates each src-block's [128, 32]
    output tile directly in PSUM across all of the block's chunks
  - one DVE copy PSUM->SBUF and one DMA writes the core's whole output
"""

import sys

sys.path.insert(0, "/opt/trn_rl_repo")

import numpy as np
import ml_dtypes

import concourse.bacc as bacc
import concourse.bass as bass
import concourse.mybir as mybir
from concourse.library_config import mlp
from concourse import bass_utils

N_NODES = 100000
D_FEAT = 32
N_CORES = 8
SRC_PER_CORE = N_NODES // N_CORES          # 12500
BLOCK = 128                                 # src nodes per block
N_BLOCKS = (SRC_PER_CORE + BLOCK - 1) // BLOCK   # 98
N_BUCKETS = 4
BUCKET = N_NODES // N_BUCKETS               # 25000 (fits int16 token index)
XPAD = 128                                  # bf16 row padded to 256B stride
NB = 64                                     # chunks per gather call / batch
NQ = 4                                      # SWDGE queues (Q7 core pairs)
NBUF = 4                                    # G/W buffer rotation depth (== NQ so
                                            # batches sharing a buffer share a queue)

LAST_RESULTS = None                         # set by kernel() for test.py


def _build_host_data(edge_index, edge_attr):
    src = np.asarray(edge_index[0], dtype=np.int64)
    dst = np.asarray(edge_index[1], dtype=np.int64)
    w = np.asarray(edge_attr, dtype=np.float32)
    E = src.shape[0]

    core = src // SRC_PER_CORE
    sloc = src % SRC_PER_CORE
    b = sloc // BLOCK
    srcl = sloc - b * BLOCK                  # 0..127 within block
    q = dst // BUCKET
    dstl = dst - q * BUCKET                  # 0..24999 within bucket

    # per (core, q, b) cell counts -> shared uniform chunk counts K[q, b]
    cell = (core * N_BUCKETS + q) * N_BLOCKS + b
    counts = np.bincount(cell, minlength=N_CORES * N_BUCKETS * N_BLOCKS)
    counts = counts.reshape(N_CORES, N_BUCKETS, N_BLOCKS)
    K = -(-counts.max(axis=0) // BLOCK)      # [N_BUCKETS, N_BLOCKS]
    K[0] = np.maximum(K[0], 1)               # every block writes its PSUM region

    chunk_start = np.zeros((N_BUCKETS, N_BLOCKS), dtype=np.int64)
    flat = K.reshape(-1)
    chunk_start.reshape(-1)[1:] = np.cumsum(flat)[:-1]
    C = int(flat.sum())

    # schedule metadata per chunk: bucket, block, start, stop
    chunk_q = np.repeat(np.arange(N_BUCKETS)[:, None], N_BLOCKS, 1).reshape(-1)
    chunk_q = np.repeat(chunk_q, flat)
    chunk_b = np.repeat(np.tile(np.arange(N_BLOCKS), N_BUCKETS), flat)
    is_start = np.zeros(C, dtype=bool)
    is_stop = np.zeros(C, dtype=bool)
    for bb in range(N_BLOCKS):
        own = np.where(chunk_b == bb)[0]
        is_start[own[0]] = True
        is_stop[own[-1]] = True

    # per-core slot assignment (slot = chunk*128 + lane)
    order = np.argsort(cell, kind="stable")
    cs = np.bincount(cell, minlength=N_CORES * N_BUCKETS * N_BLOCKS)
    cell_first = np.zeros_like(cs)
    cell_first[1:] = np.cumsum(cs)[:-1]
    rank = np.arange(E) - cell_first[cell[order]]
    slot_base = (chunk_start[q[order], b[order]] * BLOCK)
    slot = slot_base + rank                 # within this edge's core

    per_core = []
    dstl_o = dstl[order]
    srcl_o = srcl[order]
    w_o = w[order]
    core_o = core[order]
    for c in range(N_CORES):
        m = core_o == c
        s = slot[m]
        dl = np.zeros(C * BLOCK, dtype=np.int16)
        sl = np.zeros(C * BLOCK, dtype=np.int16)
        wv = np.zeros(C * BLOCK, dtype=np.float32)
        dl[s] = dstl_o[m].astype(np.int16)
        sl[s] = srcl_o[m].astype(np.int16)
        wv[s] = w_o[m]
        per_core.append((dl, sl, wv))

    # batches: per bucket, runs of <= NB chunks
    batches = []   # (q, cs_chunk, n_chunks)
    pos = 0
    for qq in range(N_BUCKETS):
        nq = int(K[qq].sum())
        done = 0
        while done < nq:
            n = min(NB, nq - done)
            batches.append((qq, pos + done, n))
            done += n
        pos += nq

    # wrapped int16 gather index arrays per core: [128, C*8]
    idx_w_cores = []
    for c in range(N_CORES):
        dl = per_core[c][0]
        cols = []
        for (qq, cs_c, n) in batches:
            flat_idx = dl[cs_c * BLOCK:(cs_c + n) * BLOCK]     # slot order == j order
            wrapped = flat_idx.reshape(-1, 16).T               # [16, ni/16]
            cols.append(np.tile(wrapped, (8, 1)))              # [128, ni/16]
        idx_w_cores.append(np.concatenate(cols, axis=1))

    sched = {
        "C": C,
        "chunk_b": chunk_b,
        "is_start": is_start,
        "is_stop": is_stop,
        "batches": batches,
    }
    return sched, per_core, idx_w_cores


def _dma_gather_raw(gpsimd, nc, out_ap, in_ap, idxs_ap, num_idxs, elem_size,
                    stride_bytes_256, queue_num=0):
    """dma_gather with a sub-256B payload (elem_size*dtype < 256B) and an
    explicit 256B-multiple row stride. Same instruction the stock wrapper
    emits; the stock wrapper just over-asserts elem alignment."""
    _in_ap = gpsimd.lower_ap_dma(in_ap, for_custom_bir_dma=True)
    _idxs_ap = gpsimd.lower_ap(idxs_ap)
    _out_ap = gpsimd.lower_ap(out_ap)
    return gpsimd.add_instruction(
        mybir.InstDMAGatherAnt(
            name=nc.get_next_instruction_name(),
            ins=[*_in_ap, _idxs_ap, gpsimd.lower_val_access(gpsimd.to_reg(num_idxs))],
            outs=[_out_ap],
            transpose=False, num_idxs=num_idxs, elem_size=elem_size,
            stride_bytes_256=stride_bytes_256, gen_mode=0, single_packet=False,
            queue_num=queue_num, sbuf_tokens_per_rank=0, sbuf_free_dim_per_rank=0,
            sbuf_free_dim_pad_per_rank=0, sbuf_byte_offset=0,
        )
    )


def _build_program(sched):
    C = sched["C"]
    chunk_b = sched["chunk_b"]
    is_start = sched["is_start"]
    is_stop = sched["is_stop"]
    batches = sched["batches"]
    nbatches = len(batches)
    OUTC = N_BLOCKS * D_FEAT                 # 3136

    bf16 = mybir.dt.bfloat16
    f32 = mybir.dt.float32

    nc = bacc.Bacc("TRN2", target_bir_lowering=False, debug=False,
                   num_devices=N_CORES, num_swdge_queues=NQ)
    x_d = nc.dram_tensor("x", [N_NODES, XPAD], bf16, kind="ExternalInput")
    idx_d = nc.dram_tensor("idxw", [128, C * 8], mybir.dt.int16, kind="ExternalInput")
    srcl_d = nc.dram_tensor("srcl", [128, C], bf16, kind="ExternalInput")
    w_d = nc.dram_tensor("w", [128, C], bf16, kind="ExternalInput")
    iota_d = nc.dram_tensor("iota", [128, 128], bf16, kind="ExternalInput")
    out_d = nc.dram_tensor("out", [128, OUTC], f32, kind="ExternalOutput")

    with (
        nc.Block() as block,
        nc.sbuf_tensor("idx_sb", [128, C * 8], mybir.dt.int16) as idx_sb,
        nc.sbuf_tensor("srcl_sb", [128, C], bf16) as srcl_sb,
        nc.sbuf_tensor("w_sb", [128, C], bf16) as w_sb,
        nc.sbuf_tensor("iota_sb", [128, 128], bf16) as iota_sb,
        nc.sbuf_tensor("g_sb", [128, NBUF, NB * D_FEAT], bf16) as g_sb,
        nc.sbuf_tensor("wm_sb", [128, NBUF, NB * 128], bf16) as wm_sb,
        nc.sbuf_tensor("out_sb", [128, OUTC], f32) as out_sb,
        nc.psum_tensor("ps", [128, OUTC], f32) as ps,
        nc.semaphore("io") as io,
        nc.semaphore("gsem0") as gsem0,
        nc.semaphore("gsem1") as gsem1,
        nc.semaphore("gsem2") as gsem2,
        nc.semaphore("gsem3") as gsem3,
        nc.semaphore("wsem") as wsem,
        nc.semaphore("psem") as psem,
        nc.semaphore("fin") as fin,
    ):
        gsems = [gsem0, gsem1, gsem2, gsem3]
        @block.sync
        def _(sync):
            sync.dma_start(idx_sb[:], idx_d[:]).then_inc(io, 16)
            sync.dma_start(srcl_sb[:], srcl_d[:]).then_inc(io, 16)
            sync.dma_start(w_sb[:], w_d[:]).then_inc(io, 16)
            sync.dma_start(iota_sb[:], iota_d[:]).then_inc(io, 16)
            sync.wait_ge(fin, 1)
            sync.dma_start(out_d[:], out_sb[:]).then_inc(io, 16)
            sync.wait_ge(io, 80)

        @block.gpsimd
        def _(gpsimd):
            gpsimd.load_library(mlp)
            gpsimd.wait_ge(io, 64)  # all inputs loaded
            icol = 0
            for i, (qq, cs_c, n) in enumerate(batches):
                if i >= NBUF:
                    gpsimd.wait_ge(psem, i - NBUF + 1)
                ni = n * BLOCK
                buf = i % NBUF
                _dma_gather_raw(
                    gpsimd, nc,
                    out_ap=g_sb[:, buf, :n * D_FEAT].rearrange(
                        "p (n e) -> p n e", e=D_FEAT),
                    in_ap=x_d[qq * BUCKET:(qq + 1) * BUCKET, :D_FEAT],
                    idxs_ap=idx_sb[:, icol:icol + ni // 16],
                    num_idxs=ni, elem_size=D_FEAT,
                    stride_bytes_256=(XPAD * 2) // 256,
                    queue_num=i % NQ,
                ).then_inc(gsems[buf], 16)
                icol += ni // 16

        @block.vector
        def _(vector):
            vector.memset(ps[:], 0.0).then_inc(wsem, 1)
            vector.wait_ge(io, 64)
            for i, (qq, cs_c, n) in enumerate(batches):
                buf = i % NBUF
                if i >= NBUF:
                    vector.wait_ge(psem, i - NBUF + 1)
                w3 = wm_sb[:, buf, :n * 128].rearrange("p (n s) -> p n s", s=128)
                vector.tensor_tensor(
                    out=w3,
                    in0=iota_sb[:, None, :].broadcast_to([128, n, 128]),
                    in1=srcl_sb[:, cs_c:cs_c + n, None].broadcast_to([128, n, 128]),
                    op=mybir.AluOpType.is_equal,
                ).then_inc(wsem, 1)
                vector.wait_ge(gsems[buf], 16 * (i // NBUF + 1))
                g3 = g_sb[:, buf, :n * D_FEAT].rearrange("p (n e) -> p n e", e=D_FEAT)
                vector.tensor_tensor(
                    out=g3, in0=g3,
                    in1=w_sb[:, cs_c:cs_c + n, None].broadcast_to([128, n, D_FEAT]),
                    op=mybir.AluOpType.mult,
                ).then_inc(wsem, 1)
            vector.wait_ge(psem, nbatches)
            vector.tensor_copy(out=out_sb[:], in_=ps[:]).then_inc(fin, 1)

        @block.tensor
        def _(tensor):
            for i, (qq, cs_c, n) in enumerate(batches):
                buf = i % NBUF
                tensor.wait_ge(wsem, 2 * (i + 1) + 1)
                for k in range(n):
                    c = cs_c + k
                    off = int(chunk_b[c]) * D_FEAT
                    mm = nc.tensor.matmul(
                        out=ps[:, off:off + D_FEAT],
                        lhsT=wm_sb[:, buf, k * 128:(k + 1) * 128],
                        rhs=g_sb[:, buf, k * D_FEAT:(k + 1) * D_FEAT],
                        start=False, stop=False,
                        skip_group_check=True,
                    )
                mm.then_inc(psem, 1)


    nc.compile()
    return nc


def kernel(edge_index, edge_attr, x):
    sched, per_core, idx_w_cores = _build_host_data(edge_index, edge_attr)
    C = sched["C"]

    x_bf = np.zeros((N_NODES, XPAD), dtype=ml_dtypes.bfloat16)
    x_bf[:, :D_FEAT] = np.asarray(x, dtype=np.float32).astype(ml_dtypes.bfloat16)
    iota = np.tile(np.arange(128, dtype=np.float32).astype(ml_dtypes.bfloat16),
                   (128, 1))

    nc = _build_program(sched)

    in_maps = []
    for c in range(N_CORES):
        dl, sl, wv = per_core[c]
        in_maps.append({
            "x": x_bf,
            "idxw": idx_w_cores[c],
            "srcl": sl.reshape(C, BLOCK).T.astype(ml_dtypes.bfloat16).copy(),
            "w": wv.reshape(C, BLOCK).T.astype(ml_dtypes.bfloat16).copy(),
            "iota": iota,
        })

    res = bass_utils.run_bass_kernel_spmd(nc, in_maps, core_ids=list(range(N_CORES)))
    global LAST_RESULTS
    LAST_RESULTS = res

    out = np.empty((N_NODES, D_FEAT), dtype=np.float32)
    for c in range(N_CORES):
        o = res.results[c]["out"]                      # [128, 98*32]
        o = o.reshape(128, N_BLOCKS, D_FEAT).transpose(1, 0, 2).reshape(-1, D_FEAT)
        out[c * SRC_PER_CORE:(c + 1) * SRC_PER_CORE] = o[:SRC_PER_CORE]
    return out



# revision 13
# speedup vs baseline: 4.7696x; 2.2354x over previous
"""GNN message passing (out = A @ x, A[src,dst] = edge_attr) on 8 TRN2 NeuronCores.

Strategy: shard by src (output segment) across 8 cores. Host assigns src nodes
to (core, block, lane) slots with a balance heuristic so the per-(bucket,block)
edge-cell counts stay close to multiples of 128 (minimizes chunk padding and
the cross-core max that a single SPMD program must cover). Per core:
  - edges are binned into (src-block [128 lanes], dst-bucket [25,000 nodes])
    cells; uniform chunk counts per cell (max over cores) make one static
    program serve all 8 cores (SPMD)
  - x rows are fetched with the SWDGE dma_gather custom op (bf16, 64B payload,
    256B stride, int16 indices local to the dst bucket), round-robined over
    4 SWDGE queues so all 4 Q7 core pairs generate descriptors in parallel
  - DVE builds a one-hot scatter matrix per 128-edge chunk (iota == src_lane)
    and scales the gathered rows by edge weight into a separate buffer
  - PE matmul (one-hot^T @ messages) accumulates each block's [128, 32]
    output tile directly in PSUM across all of the block's chunks
  - one DVE copy PSUM->SBUF and one DMA writes the core's whole output
"""

import sys

sys.path.insert(0, "/opt/trn_rl_repo")

import numpy as np
import ml_dtypes

import concourse.bacc as bacc
import concourse.bass as bass
import concourse.mybir as mybir
from concourse.library_config import mlp
from concourse import bass_utils

N_NODES = 100000
D_FEAT = 32
N_CORES = 8
SRC_PER_CORE = N_NODES // N_CORES          # 12500
BLOCK = 128                                 # src nodes per block
N_BLOCKS = (SRC_PER_CORE + BLOCK - 1) // BLOCK   # 98
N_BINS = N_CORES * N_BLOCKS                 # 784
N_BUCKETS = 4
BUCKET = N_NODES // N_BUCKETS               # 25000 (fits int16 token index)
XPAD = 128                                  # bf16 row padded to 256B stride
NB = 32                                     # chunks per gather call / batch
NQ = 4                                      # SWDGE queues (Q7 core pairs)
NBUF = 8                                    # buffer rotation depth (multiple of
                                            # NQ so batches sharing a buffer
                                            # share a queue -> ordered sem)
CAP = 4 * BLOCK                             # per-cell target (4 chunks)

LAST_RESULTS = None                         # set by kernel() for test.py


NTALL = 18  # per bucket, block positions allowed 5 chunks (rest 4)


def _position_caps():
    """cap[q, pos]: edge capacity of cell (q, pos). Position p is 'tall'
    (5 chunks) for bucket p%4 when p//4 < NTALL, else 4 chunks."""
    cap = np.full((N_BUCKETS, N_BLOCKS), 4 * BLOCK, np.int64)
    for p in range(N_BLOCKS):
        if p // 4 < NTALL and p % 4 < N_BUCKETS:
            cap[p % 4, p] = 5 * BLOCK
    return cap


def _assign_nodes(d):
    """Assign nodes to (core, block, lane) respecting per-(bucket, position)
    edge-count caps so the SPMD chunk schedule has minimal padding.

    d: [N_NODES, N_BUCKETS] per-node out-degree split by dst bucket.
    Returns bin_of_node [N], lane_of_node [N]  (bin = core * N_BLOCKS + pos).
    """
    cap_qp = _position_caps()                       # [4, 98]
    cap = np.tile(cap_qp.T, (N_CORES, 1)).astype(np.float64)   # [784, 4]
    fill = np.zeros((N_BINS, N_BUCKETS), np.float64)
    bin_of_node = np.empty(N_NODES, np.int32)
    lane_of_node = np.empty(N_NODES, np.int32)

    order = np.argsort(-d.sum(1), kind="stable")
    BIG = 1e9
    used = np.zeros(N_BINS, np.float64)
    for k, v in enumerate(order):
        r, j = divmod(k, N_BINS)
        if j == 0:
            used[:] = 0.0
        ratio = ((fill + d[v]) / cap).max(1)
        b = int(np.argmin(ratio + used))
        bin_of_node[v] = b
        lane_of_node[v] = r
        fill[b] += d[v]
        used[b] = BIG
    return bin_of_node, lane_of_node


def _build_host_data(edge_index, edge_attr):
    src = np.asarray(edge_index[0], dtype=np.int64)
    dst = np.asarray(edge_index[1], dtype=np.int64)
    w = np.asarray(edge_attr, dtype=np.float32)
    E = src.shape[0]

    q = dst // BUCKET
    dstl = (dst - q * BUCKET).astype(np.int64)

    d = np.bincount(src * N_BUCKETS + q,
                    minlength=N_NODES * N_BUCKETS).reshape(N_NODES, N_BUCKETS)
    bin_of_node, lane_of_node = _assign_nodes(d)

    ebin = bin_of_node[src]                      # [E]
    core = ebin // N_BLOCKS
    b = (ebin % N_BLOCKS).astype(np.int64)
    srcl = lane_of_node[src].astype(np.int64)    # 0..127 lane within block

    # per (core, q, b) cell counts -> shared uniform chunk counts K[q, b]
    cell = (core * N_BUCKETS + q) * N_BLOCKS + b
    counts = np.bincount(cell, minlength=N_CORES * N_BUCKETS * N_BLOCKS)
    counts = counts.reshape(N_CORES, N_BUCKETS, N_BLOCKS)
    K = -(-counts.max(axis=0) // BLOCK)          # [N_BUCKETS, N_BLOCKS]
    K[0] = np.maximum(K[0], 1)                   # every block writes its PSUM region

    chunk_start = np.zeros((N_BUCKETS, N_BLOCKS), dtype=np.int64)
    flat = K.reshape(-1)
    chunk_start.reshape(-1)[1:] = np.cumsum(flat)[:-1]
    C = int(flat.sum())

    # schedule metadata per chunk: bucket, block
    chunk_b = np.repeat(np.tile(np.arange(N_BLOCKS), N_BUCKETS), flat)

    # per-core slot assignment (slot = chunk*128 + lane)
    order = np.argsort(cell, kind="stable")
    cs = np.bincount(cell, minlength=N_CORES * N_BUCKETS * N_BLOCKS)
    cell_first = np.zeros_like(cs)
    cell_first[1:] = np.cumsum(cs)[:-1]
    rank = np.arange(E) - cell_first[cell[order]]
    slot_base = (chunk_start[q[order], b[order]] * BLOCK)
    slot = slot_base + rank                      # within this edge's core

    per_core = []
    dstl_o = dstl[order]
    srcl_o = srcl[order]
    w_o = w[order]
    core_o = core[order]
    for c in range(N_CORES):
        m = core_o == c
        s = slot[m]
        dl = np.zeros(C * BLOCK, dtype=np.int16)
        sl = np.zeros(C * BLOCK, dtype=np.int16)
        wv = np.zeros(C * BLOCK, dtype=np.float32)
        dl[s] = dstl_o[m].astype(np.int16)
        sl[s] = srcl_o[m].astype(np.int16)
        wv[s] = w_o[m]
        per_core.append((dl, sl, wv))

    # batches: per bucket, runs of <= NB chunks
    batches = []   # (q, cs_chunk, n_chunks)
    pos = 0
    for qq in range(N_BUCKETS):
        nq = int(K[qq].sum())
        done = 0
        while done < nq:
            n = min(NB, nq - done)
            batches.append((qq, pos + done, n))
            done += n
        pos += nq

    # wrapped int16 gather index arrays per core: [128, C*8]
    idx_w_cores = []
    for c in range(N_CORES):
        dl = per_core[c][0]
        cols = []
        for (qq, cs_c, n) in batches:
            flat_idx = dl[cs_c * BLOCK:(cs_c + n) * BLOCK]     # slot order == j order
            wrapped = flat_idx.reshape(-1, 16).T               # [16, ni/16]
            cols.append(np.tile(wrapped, (8, 1)))              # [128, ni/16]
        idx_w_cores.append(np.concatenate(cols, axis=1))

    sched = {
        "C": C,
        "chunk_b": chunk_b,
        "batches": batches,
    }
    node_pos = (bin_of_node, lane_of_node)
    return sched, per_core, idx_w_cores, node_pos


def _dma_gather_raw(gpsimd, nc, out_ap, in_ap, idxs_ap, num_idxs, elem_size,
                    stride_bytes_256, queue_num=0):
    """dma_gather with a sub-256B payload (elem_size*dtype < 256B) and an
    explicit 256B-multiple row stride. Same instruction the stock wrapper
    emits; the stock wrapper just over-asserts elem alignment."""
    _in_ap = gpsimd.lower_ap_dma(in_ap, for_custom_bir_dma=True)
    _idxs_ap = gpsimd.lower_ap(idxs_ap)
    _out_ap = gpsimd.lower_ap(out_ap)
    return gpsimd.add_instruction(
        mybir.InstDMAGatherAnt(
            name=nc.get_next_instruction_name(),
            ins=[*_in_ap, _idxs_ap, gpsimd.lower_val_access(gpsimd.to_reg(num_idxs))],
            outs=[_out_ap],
            transpose=False, num_idxs=num_idxs, elem_size=elem_size,
            stride_bytes_256=stride_bytes_256, gen_mode=0, single_packet=False,
            queue_num=queue_num, sbuf_tokens_per_rank=0, sbuf_free_dim_per_rank=0,
            sbuf_free_dim_pad_per_rank=0, sbuf_byte_offset=0,
        )
    )


def _build_program(sched):
    C = sched["C"]
    chunk_b = sched["chunk_b"]
    batches = sched["batches"]
    nbatches = len(batches)
    OUTC = N_BLOCKS * D_FEAT                 # 3136

    bf16 = mybir.dt.bfloat16
    f32 = mybir.dt.float32

    nc = bacc.Bacc("TRN2", target_bir_lowering=False, debug=False,
                   num_devices=N_CORES, num_swdge_queues=NQ)
    x_d = nc.dram_tensor("x", [N_NODES, XPAD], bf16, kind="ExternalInput")
    idx_d = nc.dram_tensor("idxw", [128, C * 8], mybir.dt.int16, kind="ExternalInput")
    srcl_d = nc.dram_tensor("srcl", [128, C], bf16, kind="ExternalInput")
    w_d = nc.dram_tensor("w", [128, C], bf16, kind="ExternalInput")
    iota_d = nc.dram_tensor("iota", [128, 128], bf16, kind="ExternalInput")
    out_d = nc.dram_tensor("out", [128, OUTC], f32, kind="ExternalOutput")

    from contextlib import ExitStack
    with ExitStack() as ctx:
        block = ctx.enter_context(nc.Block())
        idx_sb = ctx.enter_context(
            nc.sbuf_tensor("idx_sb", [128, C * 8], mybir.dt.int16))
        srcl_sb = ctx.enter_context(nc.sbuf_tensor("srcl_sb", [128, C], bf16))
        w_sb = ctx.enter_context(nc.sbuf_tensor("w_sb", [128, C], bf16))
        iota_sb = ctx.enter_context(nc.sbuf_tensor("iota_sb", [128, 128], bf16))
        g_sb = ctx.enter_context(
            nc.sbuf_tensor("g_sb", [128, NBUF, NB * D_FEAT], bf16))
        g2_sb = ctx.enter_context(
            nc.sbuf_tensor("g2_sb", [128, NBUF, NB * D_FEAT], bf16))
        wm_sb = ctx.enter_context(
            nc.sbuf_tensor("wm_sb", [128, NBUF, NB * 128], bf16))
        out_sb = ctx.enter_context(nc.sbuf_tensor("out_sb", [128, OUTC], f32))
        ps = ctx.enter_context(nc.psum_tensor("ps", [128, OUTC], f32))
        io = ctx.enter_context(nc.semaphore("io"))
        gsems = [ctx.enter_context(nc.semaphore(f"gsem{i}")) for i in range(NBUF)]
        wsem = ctx.enter_context(nc.semaphore("wsem"))
        psem = ctx.enter_context(nc.semaphore("psem"))
        fin = ctx.enter_context(nc.semaphore("fin"))

        @block.sync
        def _(sync):
            sync.dma_start(idx_sb[:], idx_d[:]).then_inc(io, 16)
            sync.dma_start(srcl_sb[:], srcl_d[:]).then_inc(io, 16)
            sync.dma_start(w_sb[:], w_d[:]).then_inc(io, 16)
            sync.dma_start(iota_sb[:], iota_d[:]).then_inc(io, 16)
            sync.wait_ge(fin, 1)
            sync.dma_start(out_d[:], out_sb[:]).then_inc(io, 16)
            sync.wait_ge(io, 80)

        @block.gpsimd
        def _(gpsimd):
            gpsimd.load_library(mlp)
            gpsimd.wait_ge(io, 64)  # all inputs loaded
            icol = 0
            for i, (qq, cs_c, n) in enumerate(batches):
                if i >= NBUF:
                    # scale of batch i-NBUF has fully read g_sb[buf]
                    gpsimd.wait_ge(wsem, 2 * (i - NBUF) + 3)
                ni = n * BLOCK
                buf = i % NBUF
                _dma_gather_raw(
                    gpsimd, nc,
                    out_ap=g_sb[:, buf, :n * D_FEAT].rearrange(
                        "p (n e) -> p n e", e=D_FEAT),
                    in_ap=x_d[qq * BUCKET:(qq + 1) * BUCKET, :D_FEAT],
                    idxs_ap=idx_sb[:, icol:icol + ni // 16],
                    num_idxs=ni, elem_size=D_FEAT,
                    stride_bytes_256=(XPAD * 2) // 256,
                    queue_num=i % NQ,
                ).then_inc(gsems[buf], 16)
                icol += ni // 16

        @block.vector
        def _(vector):
            vector.memset(ps[:], 0.0).then_inc(wsem, 1)
            vector.wait_ge(io, 64)
            for i, (qq, cs_c, n) in enumerate(batches):
                buf = i % NBUF
                if i >= NBUF:
                    # matmuls of batch i-NBUF have read wm_sb/g2_sb[buf]
                    vector.wait_ge(psem, i - NBUF + 1)
                w3 = wm_sb[:, buf, :n * 128].rearrange("p (n s) -> p n s", s=128)
                vector.tensor_tensor(
                    out=w3,
                    in0=iota_sb[:, None, :].broadcast_to([128, n, 128]),
                    in1=srcl_sb[:, cs_c:cs_c + n, None].broadcast_to([128, n, 128]),
                    op=mybir.AluOpType.is_equal,
                ).then_inc(wsem, 1)
                vector.wait_ge(gsems[buf], 16 * (i // NBUF + 1))
                g3 = g_sb[:, buf, :n * D_FEAT].rearrange("p (n e) -> p n e", e=D_FEAT)
                g4 = g2_sb[:, buf, :n * D_FEAT].rearrange("p (n e) -> p n e", e=D_FEAT)
                vector.tensor_tensor(
                    out=g4, in0=g3,
                    in1=w_sb[:, cs_c:cs_c + n, None].broadcast_to([128, n, D_FEAT]),
                    op=mybir.AluOpType.mult,
                ).then_inc(wsem, 1)
            vector.wait_ge(psem, nbatches)
            vector.tensor_copy(out=out_sb[:], in_=ps[:]).then_inc(fin, 1)

        @block.tensor
        def _(tensor):
            for i, (qq, cs_c, n) in enumerate(batches):
                buf = i % NBUF
                tensor.wait_ge(wsem, 2 * (i + 1) + 1)
                for k in range(n):
                    c = cs_c + k
                    off = int(chunk_b[c]) * D_FEAT
                    mm = nc.tensor.matmul(
                        out=ps[:, off:off + D_FEAT],
                        lhsT=wm_sb[:, buf, k * 128:(k + 1) * 128],
                        rhs=g2_sb[:, buf, k * D_FEAT:(k + 1) * D_FEAT],
                        start=False, stop=False,
                        skip_group_check=True,
                    )
                mm.then_inc(psem, 1)


    nc.compile()
    return nc


def kernel(edge_index, edge_attr, x):
    sched, per_core, idx_w_cores, node_pos = _build_host_data(edge_index, edge_attr)
    C = sched["C"]

    x_bf = np.zeros((N_NODES, XPAD), dtype=ml_dtypes.bfloat16)
    x_bf[:, :D_FEAT] = np.asarray(x, dtype=np.float32).astype(ml_dtypes.bfloat16)
    iota = np.tile(np.arange(128, dtype=np.float32).astype(ml_dtypes.bfloat16),
                   (128, 1))

    nc = _build_program(sched)

    in_maps = []
    for c in range(N_CORES):
        dl, sl, wv = per_core[c]
        in_maps.append({
            "x": x_bf,
            "idxw": idx_w_cores[c],
            "srcl": sl.reshape(C, BLOCK).T.astype(ml_dtypes.bfloat16).copy(),
            "w": wv.reshape(C, BLOCK).T.astype(ml_dtypes.bfloat16).copy(),
            "iota": iota,
        })

    res = bass_utils.run_bass_kernel_spmd(nc, in_maps, core_ids=list(range(N_CORES)))
    global LAST_RESULTS
    LAST_RESULTS = res

    bin_of_node, lane_of_node = node_pos
    out = np.empty((N_NODES, D_FEAT), dtype=np.float32)
    core_of_node = bin_of_node // N_BLOCKS
    block_of_node = bin_of_node % N_BLOCKS
    for c in range(N_CORES):
        o = res.results[c]["out"]                      # [128, 98*32]
        o = o.reshape(128, N_BLOCKS, D_FEAT)
        m = core_of_node == c
        out[np.where(m)[0]] = o[lane_of_node[m], block_of_node[m]]
    return out
